# revision 22
# baseline (speedup 1.0000x reference)
"""Trainium2 Bass kernel for a dense transformer decoder block on 8 NeuronCores.

Sharding (uniform SPMD, v2 — AllGather design):
  * tokens: core c owns 512 tokens of batch b=c//4: the two 256-position
    stripes {256*c4, 256*(7-c4)} (c4=c%4). The stripe pairing balances causal
    attention work exactly (18 kv-blocks of 128 per head on every core).
  * attention is query-sharded: each core attends its OWN 512 queries over
    ALL heads. K and V (computed locally per token owner, rope applied to K)
    are AllGathered within the 4-core batch group — the only collectives in
    the kernel, each fired right after its producing projection so it hides
    under the next projection's compute. Q, attention output, the
    O-projection, residual, norm2 and the FFN are all local. No AllToAll,
    no all-reduce.
  * SPMD uniformity: every core runs the same padded kv-prefix length per
    query slot (8 blocks for slot 0, 16 for slot 1). Per-core mask *data*
    (multiplicative {0,1} on the exp output: causal diagonal + padding,
    plus the key-padding bias inside the exp) zeroes blocks beyond that
    core's real causal extent.

Everything runs in bf16 on the PE (fp32 PSUM accumulation): 2x less HBM
traffic and collective payload than fp32, and bf16 enables fast-weight-load
so LDWEIGHTS overlaps matmuls. Weights are host-packed into the exact
column-block tile layout so every weight DMA is one contiguous 2D transfer.
Softmax statistics (exp on ScalarE, denominator via ones-matmul) and RMSNorm
statistics (Square + ones-matmul) ride on otherwise idle engines.
"""
import sys

sys.path.insert(0, '/opt/trn_rl_repo')

import numpy as np
import ml_dtypes

import concourse.bacc as bacc
import concourse.mybir as mybir
from concourse import tile
from concourse.bass_utils import run_bass_kernel_spmd

F32 = mybir.dt.float32
F32R = mybir.dt.float32r
BF16 = mybir.dt.bfloat16
AF = mybir.ActivationFunctionType

D = 2048
H = 16
DH = 128
FF = 8192
B = 2
L = 2048
NCORES = 8
TOK = 512            # tokens per core
NF = D // 128        # 16 feature chunks
NEG = -30000.0
EPS = float(np.finfo(np.float32).eps)
ISQ = 1.0 / float(np.sqrt(DH))
RG4 = [[0, 1, 2, 3], [4, 5, 6, 7]]


def _kv_loc(j):
    """kv 128-block j (absolute) -> (rank, slot, half) in gathered buffers."""
    p, hf = j // 2, j % 2
    r = p if p < 4 else 7 - p
    s = 0 if p < 4 else 1
    return r, s, hf


def _build():
    nc = bacc.Bacc("TRN2", target_bir_lowering=False, debug=False,
                   num_devices=NCORES)

    xT = nc.dram_tensor("xT", [D, TOK], F32, kind="ExternalInput")
    wqp = nc.dram_tensor("wqp", [NF * 128, D], BF16, kind="ExternalInput")
    wkp = nc.dram_tensor("wkp", [NF * 128, D], BF16, kind="ExternalInput")
    wvp = nc.dram_tensor("wvp", [8 * 128, 4096], BF16, kind="ExternalInput")
    wop = nc.dram_tensor("wop", [NF * 128, D], BF16, kind="ExternalInput")
    wf1p = nc.dram_tensor("wf1p", [64 * 128, D], BF16, kind="ExternalInput")
    wf2p = nc.dram_tensor("wf2p", [NF * 128, FF], BF16, kind="ExternalInput")
    ropeC = nc.dram_tensor("ropeC", [DH, TOK], F32, kind="ExternalInput")
    ropeS2 = nc.dram_tensor("ropeS2", [DH, TOK], F32, kind="ExternalInput")
    maskM = nc.dram_tensor("maskM", [128, 16 * 256], BF16,
                           kind="ExternalInput")
    mbias = nc.dram_tensor("mbias", [128, 16], F32, kind="ExternalInput")
    onesf = nc.dram_tensor("onesf", [128, 1], F32R, kind="ExternalInput")
    onesb = nc.dram_tensor("onesb", [128, 1], BF16, kind="ExternalInput")
    outT = nc.dram_tensor("outT", [D, TOK], F32, kind="ExternalOutput")

    # internal DRAM: AllGather bounce buffers, split in halves so the two
    # gathers per tensor pipeline on the single CC stream
    kgin1 = nc.dram_tensor("kgin1", [D // 2, TOK], BF16)   # heads 0-7
    kgout1 = nc.dram_tensor("kgout1", [2 * D, TOK], BF16)
    kgin2 = nc.dram_tensor("kgin2", [D // 2, TOK], BF16)   # heads 8-15
    kgout2 = nc.dram_tensor("kgout2", [2 * D, TOK], BF16)
    vgin1 = nc.dram_tensor("vgin1", [TOK, D // 2], BF16)   # features 0-1023
    vgout1 = nc.dram_tensor("vgout1", [4 * TOK, D // 2], BF16)
    vgin2 = nc.dram_tensor("vgin2", [TOK, D // 2], BF16)   # features 1024-
    vgout2 = nc.dram_tensor("vgout2", [4 * TOK, D // 2], BF16)

    with tile.TileContext(nc) as tc:
        with (
            tc.tile_pool(name="const", bufs=1) as cp,
            tc.tile_pool(name="small", bufs=1) as sp,
            tc.tile_pool(name="mq", bufs=1) as mq,
        ):
            one_r = cp.tile([128, 1], F32R)
            one_b = cp.tile([128, 1], BF16)
            epsc = cp.tile([1, 1], F32)
            nc.scalar.dma_start(one_r[:], onesf[:])
            nc.scalar.dma_start(one_b[:], onesb[:])
            nc.gpsimd.memset(epsc[:], EPS)

            rsB = sp.tile([128, TOK], F32)
            rowS = sp.tile([1, TOK], F32)
            rowR = sp.tile([1, TOK], F32)
            qt = mq.tile([128, NF * TOK], BF16, tag="qt")

            def rmsnorm_rs(ssq_ps):
                # rowR = 1/sqrt(ssq/D + eps), broadcast to 128 partitions
                nc.scalar.activation(rowS[:], ssq_ps[:], AF.Sqrt,
                                     bias=epsc[:], scale=1.0 / D)
                nc.vector.reciprocal(rowR[:], rowS[:])
                nc.gpsimd.partition_broadcast(rsB[:], rowR[:])

            # ====== Phase A-D: norm1, K/V/Q projections, AllGathers ======
            with (
                tc.tile_pool(name="m1", bufs=1) as m1,
                tc.tile_pool(name="ps1", bufs=3, space="PSUM") as ps1,
                tc.tile_pool(name="psr", bufs=1, space="PSUM") as psr,
            ):
                cosT = m1.tile([DH, TOK], F32, tag="cosT")
                sin2 = m1.tile([DH, TOK], F32, tag="sin2")
                nc.scalar.dma_start(cosT[:], ropeC[:])
                nc.scalar.dma_start(sin2[:], ropeS2[:])

                xt = m1.tile([128, NF * TOK], F32, tag="xt")
                xeng = [nc.scalar, nc.gpsimd, nc.scalar, nc.gpsimd]
                for g in range(4):
                    xeng[g].dma_start(
                        xt[:, g * 4 * TOK:(g + 1) * 4 * TOK]
                        .rearrange("p (i c) -> p i c", i=4),
                        xT[g * 512:(g + 1) * 512, :]
                        .rearrange("(i p) c -> p i c", p=128))

                ssq = psr.tile([1, TOK], F32, tag="row")
                for i in range(NF):
                    sq = sp.tile([128, TOK], F32R, tag="sq", bufs=1)
                    nc.scalar.activation(sq[:], xt[:, i * TOK:(i + 1) * TOK],
                                         AF.Square)
                    nc.tensor.matmul(ssq[:], one_r[:], sq[:],
                                     start=(i == 0), stop=(i == NF - 1))
                rmsnorm_rs(ssq)
                xnt = m1.tile([128, NF * TOK], BF16, tag="xn")
                for i in range(NF):
                    nc.vector.tensor_mul(xnt[:, i * TOK:(i + 1) * TOK],
                                         xt[:, i * TOK:(i + 1) * TOK], rsB[:])

                def proj_T(wten, out_tile, rope):
                    """out_tile[:, o*TOK:] = feature-block o of (xn @ w)^T."""
                    for o in range(NF):
                        wc = m1.tile([128, D], BF16, tag="wcol", bufs=4)
                        nc.sync.dma_start(wc[:],
                                          wten[o * 128:(o + 1) * 128, :])
                        acc = ps1.tile([128, TOK], F32, tag="big")
                        for i in range(NF):
                            nc.tensor.matmul(
                                acc[:], wc[:, i * 128:(i + 1) * 128],
                                xnt[:, i * TOK:(i + 1) * TOK],
                                start=(i == 0), stop=(i == NF - 1))
                        dst = out_tile[:, o * TOK:(o + 1) * TOK]
                        if rope:
                            tmp = sp.tile([128, TOK], F32, tag="rtmp",
                                          bufs=1)
                            tmc = sp.tile([128, TOK], F32, tag="rtmc",
                                          bufs=1)
                            nc.vector.tensor_mul(tmp[0:64, :], acc[64:128, :],
                                                 sin2[0:64, :])
                            nc.vector.tensor_mul(tmp[64:128, :], acc[0:64, :],
                                                 sin2[64:128, :])
                            nc.vector.tensor_mul(tmc[:], acc[:], cosT[:])
                            nc.vector.tensor_add(dst, tmc[:], tmp[:])
                        else:
                            nc.vector.tensor_copy(dst, acc[:])
                        yield o

                # K^T (roped) -> kgin chunks -> 2 AllGathers (head halves)
                kt = m1.tile([128, NF * TOK], BF16, tag="kt")
                for o in proj_T(wkp, kt, rope=True):
                    kg = kgin1 if o < 8 else kgin2
                    oo = o % 8
                    nc.gpsimd.dma_start(kg.ap()[oo * 128:(oo + 1) * 128, :],
                                        kt[:, o * TOK:(o + 1) * TOK])
                    if o == 7:
                        nc.gpsimd.collective_compute(
                            "AllGather", mybir.AluOpType.bypass,
                            replica_groups=RG4, ins=[kgin1.ap().opt()],
                            outs=[kgout1.ap().opt()])
                nc.gpsimd.collective_compute(
                    "AllGather", mybir.AluOpType.bypass, replica_groups=RG4,
                    ins=[kgin2.ap().opt()], outs=[kgout2.ap().opt()])

                # V (token-major) -> vgin -> 2 AllGathers (feature halves)
                vt = m1.tile([128, 4 * D], BF16, tag="vt")
                for fo in range(8):
                    wvc = m1.tile([128, NF * 256], BF16, tag="wv", bufs=3)
                    nc.sync.dma_start(wvc[:],
                                      wvp[fo * 128:(fo + 1) * 128, :])
                    for to in range(4):
                        acc = ps1.tile([128, 256], F32, tag="vacc", bufs=3)
                        for i in range(NF):
                            nc.tensor.matmul(
                                acc[:],
                                xnt[:, i * TOK + to * 128:
                                    i * TOK + (to + 1) * 128],
                                wvc[:, i * 256:(i + 1) * 256],
                                start=(i == 0), stop=(i == NF - 1))
                        nc.vector.tensor_copy(
                            vt[:, to * D + fo * 256:to * D + (fo + 1) * 256],
                            acc[:])
                    if fo == 3 or fo == 7:
                        vg, base = (vgin1, 0) if fo == 3 else (vgin2, 1024)
                        for to in range(4):
                            nc.gpsimd.dma_start(
                                vg.ap()[to * 128:(to + 1) * 128, :],
                                vt[:, to * D + base:to * D + base + 1024])
                        nc.gpsimd.collective_compute(
                            "AllGather", mybir.AluOpType.bypass,
                            replica_groups=RG4,
                            ins=[vg.ap().opt()],
                            outs=[(vgout1 if fo == 3 else vgout2).ap().opt()])

                # Q^T (roped), stays local in qt
                for _ in proj_T(wqp, qt, rope=True):
                    pass

            # ============ Phase E: attention (local queries) =============
            with tc.tile_pool(name="mo", bufs=1) as mo:
                osb = mo.tile([128, NF * TOK], BF16, tag="osb")
                with (
                    tc.tile_pool(name="m2", bufs=1) as m2,
                    tc.tile_pool(name="ps_s", bufs=4, space="PSUM") as ps_s,
                    tc.tile_pool(name="ps_av", bufs=2, space="PSUM") as ps_av,
                    tc.tile_pool(name="ps_dn", bufs=2, space="PSUM") as ps_dn,
                ):
                    mkT = m2.tile([128, 16 * 256], BF16, tag="mkT")
                    mbT = m2.tile([128, 16], F32, tag="mbT")
                    nc.scalar.dma_start(mkT[:], maskM[:])
                    nc.scalar.dma_start(mbT[:], mbias[:])

                    # ksb view: [dh, h, r, c(512)] ; vsb view: [p, G, f]
                    ksb = m2.tile([128, 16 * D], BF16, tag="ksb")
                    vsb = m2.tile([128, 16 * D], BF16, tag="vsb")
                    ksbv = ksb[:].rearrange("d (h r c) -> d h r c", h=16, r=4)
                    vsbv = vsb[:].rearrange("p (g f) -> p g f", g=16)
                    eng4 = [nc.scalar, nc.sync, nc.gpsimd, nc.scalar]
                    for r in range(4):
                        eng4[r].dma_start(
                            ksbv[:, 0:8, r, :],
                            kgout1.ap()[r * 1024:(r + 1) * 1024, :]
                            .rearrange("(h d) c -> d h c", d=128))
                        eng4[(r + 1) % 4].dma_start(
                            ksbv[:, 8:16, r, :],
                            kgout2.ap()[r * 1024:(r + 1) * 1024, :]
                            .rearrange("(h d) c -> d h c", d=128))
                        eng4[(r + 2) % 4].dma_start(
                            vsbv[:, r * 4:(r + 1) * 4, 0:1024],
                            vgout1.ap()[r * TOK:(r + 1) * TOK, :]
                            .rearrange("(g p) f -> p g f", p=128))
                        eng4[(r + 3) % 4].dma_start(
                            vsbv[:, r * 4:(r + 1) * 4, 1024:2048],
                            vgout2.ap()[r * TOK:(r + 1) * TOK, :]
                            .rearrange("(g p) f -> p g f", p=128))

                    PIPE = 3
                    for h in range(H):
                        for s in range(2):
                            qs = qt[:, h * TOK + s * 256:
                                    h * TOK + (s + 1) * 256]
                            n_it = 8 if s == 0 else 16
                            av = ps_av.tile([128, 256], F32, tag="av")
                            dn = ps_dn.tile([1, 256], F32, tag="dn")
                            pts = [None] * n_it
                            # software pipeline: scores+exp run PIPE iters
                            # ahead of the dn/av accumulation matmuls
                            for u in range(n_it + PIPE):
                                if u < n_it:
                                    t = u
                                    r, sl, hf = _kv_loc(t)
                                    kcol = (h * D + r * TOK + sl * 256
                                            + hf * 128)
                                    st = ps_s.tile([128, 256], F32, tag="st")
                                    nc.tensor.matmul(
                                        st[:], ksb[:, kcol:kcol + 128], qs,
                                        start=True, stop=True)
                                    pt = sp.tile([128, 256], BF16, tag="pt",
                                                 bufs=PIPE + 2)
                                    nc.scalar.activation(
                                        pt[:], st[:], AF.Exp,
                                        bias=mbT[:, t:t + 1], scale=ISQ)
                                    if s == 0 or t >= 8:
                                        nc.vector.tensor_mul(
                                            pt[:], pt[:],
                                            mkT[:, t * 256:(t + 1) * 256])
                                    pts[t] = pt
                                if u >= PIPE:
                                    t = u - PIPE
                                    r, sl, hf = _kv_loc(t)
                                    G = r * 4 + sl * 2 + hf
                                    nc.tensor.matmul(dn[:], one_b[:],
                                                     pts[t][:],
                                                     start=(t == 0),
                                                     stop=(t == n_it - 1))
                                    nc.tensor.matmul(
                                        av[:], vsb[:, G * D + h * 128:
                                                   G * D + (h + 1) * 128],
                                        pts[t][:], start=(t == 0),
                                        stop=(t == n_it - 1))
                            dnr = sp.tile([1, 256], F32, tag="dnr", bufs=2)
                            nc.scalar.activation(dnr[:], dn[:], AF.Copy)
                            dnB = sp.tile([128, 256], F32, tag="dnB", bufs=2)
                            nc.gpsimd.partition_broadcast(dnB[:], dnr[:])
                            rdB = sp.tile([128, 256], F32, tag="rdB", bufs=2)
                            nc.vector.reciprocal(rdB[:], dnB[:])
                            nc.vector.tensor_mul(
                                osb[:, h * TOK + s * 256:
                                    h * TOK + (s + 1) * 256],
                                av[:], rdB[:])

                # ===== Phase F: O-projection + residual + norm2 stats ====
                with (
                    tc.tile_pool(name="m3", bufs=1) as m3,
                    tc.tile_pool(name="ps3", bufs=3, space="PSUM") as ps3,
                    tc.tile_pool(name="psr3", bufs=1, space="PSUM") as psr3,
                ):
                    x2 = m3.tile([128, NF * TOK], F32, tag="x2")
                    ssq2 = psr3.tile([1, TOK], F32, tag="row")
                    for o in range(NF):
                        wc = m3.tile([128, D], BF16, tag="wocol", bufs=2)
                        nc.sync.dma_start(wc[:],
                                          wop[o * 128:(o + 1) * 128, :])
                        acc = ps3.tile([128, TOK], F32, tag="big")
                        for i in range(NF):
                            nc.tensor.matmul(
                                acc[:], wc[:, i * 128:(i + 1) * 128],
                                osb[:, i * TOK:(i + 1) * TOK],
                                start=(i == 0), stop=(i == NF - 1))
                        xsl = m3.tile([128, TOK], F32, tag="xsl", bufs=2)
                        nc.scalar.dma_start(xsl[:],
                                            xT[o * 128:(o + 1) * 128, :])
                        nc.vector.tensor_add(x2[:, o * TOK:(o + 1) * TOK],
                                             xsl[:], acc[:])
                        sq = sp.tile([128, TOK], F32R, tag="sq", bufs=1)
                        nc.scalar.activation(sq[:],
                                             x2[:, o * TOK:(o + 1) * TOK],
                                             AF.Square)
                        nc.tensor.matmul(ssq2[:], one_r[:], sq[:],
                                         start=(o == 0), stop=(o == NF - 1))
                    rmsnorm_rs(ssq2)

                    # ================ Phase G/H: norm2 + FFN =============
                    with (
                        tc.tile_pool(name="m4", bufs=1) as m4,
                        tc.tile_pool(name="ps4", bufs=3, space="PSUM") as ps4,
                    ):
                        xn2 = m4.tile([128, NF * TOK], BF16, tag="xn2")
                        for i in range(NF):
                            nc.vector.tensor_mul(
                                xn2[:, i * TOK:(i + 1) * TOK],
                                x2[:, i * TOK:(i + 1) * TOK], rsB[:])
                        # ff1 + silu -> hb (bf16, SBUF resident)
                        hb = m4.tile([128, 64 * TOK], BF16, tag="hb")
                        for o in range(FF // 128):
                            wc = m4.tile([128, D], BF16, tag="wf1c", bufs=2)
                            nc.sync.dma_start(
                                wc[:], wf1p[o * 128:(o + 1) * 128, :])
                            acc = ps4.tile([128, TOK], F32, tag="big")
                            for i in range(NF):
                                nc.tensor.matmul(
                                    acc[:], wc[:, i * 128:(i + 1) * 128],
                                    xn2[:, i * TOK:(i + 1) * TOK],
                                    start=(i == 0), stop=(i == NF - 1))
                            nc.scalar.activation(hb[:, o * TOK:(o + 1) * TOK],
                                                 acc[:], AF.Silu)
                        # ff2 + residual -> outT (weights in 1MB half-chunks)
                        for o in range(NF):
                            acc = ps4.tile([128, TOK], F32, tag="big")
                            for half in range(2):
                                wc2 = m4.tile([128, 32 * 128], BF16,
                                              tag="wf2c", bufs=2)
                                nc.sync.dma_start(
                                    wc2[:],
                                    wf2p[o * 128:(o + 1) * 128,
                                         half * 4096:(half + 1) * 4096])
                                for kk in range(32):
                                    k = half * 32 + kk
                                    nc.tensor.matmul(
                                        acc[:],
                                        wc2[:, kk * 128:(kk + 1) * 128],
                                        hb[:, k * TOK:(k + 1) * TOK],
                                        start=(k == 0), stop=(k == 63))
                            osl = m4.tile([128, TOK], F32, tag="osl", bufs=2)
                            nc.vector.tensor_add(osl[:],
                                                 x2[:, o * TOK:(o + 1) * TOK],
                                                 acc[:])
                            nc.sync.dma_start(outT[o * 128:(o + 1) * 128, :],
                                              osl[:])

    nc.compile()
    return nc


_COMPILED = None


def _own_positions(c):
    c4 = c % 4
    a0, a1 = c4, 7 - c4
    pos = np.r_[256 * a0:256 * a0 + 256, 256 * a1:256 * a1 + 256]
    return pos, a0, a1


def _pack_cols(w, oc):
    """[K, M] -> chunk-major [no*128, nk*oc]: rows o*128+p hold, for output
    column chunk o, the weight rows (i*128+p, o-chunk) laid out (i, m)."""
    k, m = w.shape
    nk, no = k // 128, m // oc
    out = np.empty((no * 128, nk * oc), w.dtype)
    for o in range(no):
        c = w[:, o * oc:(o + 1) * oc].reshape(nk, 128, oc)
        out[o * 128:(o + 1) * 128, :] = (
            c.transpose(1, 0, 2).reshape(128, nk * oc))
    return out


def _prep_inmaps(x, rope_cos, rope_sin, mask, w_norm1, w_norm2, wq, wk, wv,
                 wo, w_ff1, w_ff2):
    x = np.asarray(x, np.float32)
    cos = np.asarray(rope_cos, np.float32)
    sin = np.asarray(rope_sin, np.float32)
    mask = np.asarray(mask)
    wn1 = np.asarray(w_norm1, np.float32)[:, None]
    wn2 = np.asarray(w_norm2, np.float32)[:, None]

    bf = ml_dtypes.bfloat16
    wqp = _pack_cols((wn1 * np.asarray(wq, np.float32)).astype(bf), 128)
    wkp = _pack_cols((wn1 * np.asarray(wk, np.float32)).astype(bf), 128)
    wvp = _pack_cols((wn1 * np.asarray(wv, np.float32)).astype(bf), 256)
    wop = _pack_cols(np.asarray(wo, np.float32).astype(bf), 128)
    wf1p = _pack_cols((wn2 * np.asarray(w_ff1, np.float32)).astype(bf), 128)
    wf2p = _pack_cols(np.asarray(w_ff2, np.float32).astype(bf), 128)

    rr = np.arange(128)[:, None]
    cc = np.arange(256)[None, :]
    diag_lo = (cc >= rr).astype(np.float32)
    diag_hi = (cc >= rr + 128).astype(np.float32)
    zeros = np.zeros((128, 256), np.float32)
    ones = np.ones((128, 256), np.float32)

    in_maps = []
    for c in range(NCORES):
        b = c // 4
        pos, a0, a1 = _own_positions(c)
        s = sin[pos].T.copy()
        s2 = np.concatenate([-s[:64], s[64:]], axis=0)

        # multiplicative mask tiles: t<8 -> slot 0 (q-block a0), else slot 1
        tiles = []
        for t in range(16):
            a = a0 if t < 8 else a1
            ext = 2 * a + 2
            if t == ext - 2:
                tiles.append(diag_lo)
            elif t == ext - 1:
                tiles.append(diag_hi)
            elif t >= ext:
                tiles.append(zeros)
            else:
                tiles.append(ones)
        maskM = np.concatenate(tiles, axis=1).astype(bf)

        # key-padding bias per kv block: col j <- kv pos 128*j + r
        mb = np.where(mask[b] != 0, 0.0, NEG).astype(np.float32)  # [L]
        mbias = np.ascontiguousarray(mb.reshape(16, 128).T)

        in_maps.append({
            "xT": np.ascontiguousarray(x[b, pos].T),
            "wqp": wqp, "wkp": wkp, "wvp": wvp, "wop": wop,
            "wf1p": wf1p, "wf2p": wf2p,
            "ropeC": np.ascontiguousarray(cos[pos].T),
            "ropeS2": np.ascontiguousarray(s2),
            "maskM": np.ascontiguousarray(maskM),
            "mbias": mbias,
            "onesf": np.ones((128, 1), np.float32),
            "onesb": np.ones((128, 1), ml_dtypes.bfloat16),
        })
    return in_maps


def _assemble(res):
    out = np.empty((B, L, D), np.float32)
    for c in range(NCORES):
        b = c // 4
        pos, _, _ = _own_positions(c)
        out[b, pos, :] = res.results[c]["outT"].T
    return out


def kernel(**inputs):
    global _COMPILED
    if _COMPILED is None:
        _COMPILED = _build()
    in_maps = _prep_inmaps(**inputs)
    res = run_bass_kernel_spmd(_COMPILED, in_maps, list(range(NCORES)))
    return _assemble(res)


def timed_run(**inputs):
    """Run with NTFF profiling; returns (exec_time_ns, BassKernelResults)."""
    global _COMPILED
    if _COMPILED is None:
        _COMPILED = _build()
    in_maps = _prep_inmaps(**inputs)
    res = run_bass_kernel_spmd(_COMPILED, in_maps, list(range(NCORES)),
                               trace=True)
    return res.exec_time_ns, res


# revision 27
# speedup vs baseline: 1.0336x; 1.0336x over previous
"""Trainium2 Bass kernel for a dense transformer decoder block on 8 NeuronCores.

Sharding (uniform SPMD, v3 — AllGather design):
  * tokens: core c owns 512 tokens of batch b=c//4: the two 256-position
    stripes {256*c4, 256*(7-c4)} (c4=c%4). The stripe pairing balances causal
    attention work exactly (18 kv-blocks of 128 per head on every core).
  * attention is query-sharded: each core attends its OWN 512 queries over
    ALL heads. K and V (computed locally per token owner, rope applied to K)
    are AllGathered within the 4-core batch group — the only collectives in
    the kernel. Each gather is split in half and fired mid-projection so the
    serial CC-stream chain (k1, v1, k2, v2) finishes as attention starts;
    heads are consumed in ascending order so the second halves land in time.
    Q, attention output, the O-projection, residual, norm2 and the FFN are
    all local. No AllToAll, no all-reduce.
  * SPMD uniformity: every core runs the same padded kv-prefix length per
    query slot (8 blocks for slot 0, 16 for slot 1). Per-core mask *data*
    (multiplicative {0,1} on the exp output + key-padding bias inside the
    exp) zeroes blocks beyond that core's real causal extent.

Scheduling notes: everything runs in bf16 on the PE (fp32 PSUM
accumulation); weights are host-packed into the exact tile layout so every
weight DMA is one contiguous 2D transfer on the sync queue. The attention
inner loop is software-pipelined (scores+exp run PIPE iterations ahead of
the dn/av accumulations) to decouple the in-order PE queue from the
ScalarE exp chain. Gathered K/V are streamed into SBUF in 4-head group
chunks: only the first group sits in the DMA-congested projection window;
later groups load during attention itself.
"""
import sys

sys.path.insert(0, '/opt/trn_rl_repo')

import numpy as np
import ml_dtypes

import concourse.bacc as bacc
import concourse.mybir as mybir
from concourse import tile
from concourse.bass_utils import run_bass_kernel_spmd

F32 = mybir.dt.float32
F32R = mybir.dt.float32r
BF16 = mybir.dt.bfloat16
AF = mybir.ActivationFunctionType

D = 2048
H = 16
DH = 128
FF = 8192
B = 2
L = 2048
NCORES = 8
TOK = 512            # tokens per core
NF = D // 128        # 16 feature chunks
NEG = -30000.0
EPS = float(np.finfo(np.float32).eps)
ISQ = 1.0 / float(np.sqrt(DH))
RG4 = [[0, 1, 2, 3], [4, 5, 6, 7]]


def _kv_loc(j):
    """kv 128-block j (absolute) -> (rank, slot, half) in gathered buffers."""
    p, hf = j // 2, j % 2
    r = p if p < 4 else 7 - p
    s = 0 if p < 4 else 1
    return r, s, hf


def _build():
    nc = bacc.Bacc("TRN2", target_bir_lowering=False, debug=False,
                   num_devices=NCORES)

    xT = nc.dram_tensor("xT", [D, TOK], F32, kind="ExternalInput")
    wqp = nc.dram_tensor("wqp", [NF * 128, D], BF16, kind="ExternalInput")
    wkp = nc.dram_tensor("wkp", [NF * 128, D], BF16, kind="ExternalInput")
    wvp = nc.dram_tensor("wvp", [4 * 128, NF * 512], BF16,
                         kind="ExternalInput")
    wop = nc.dram_tensor("wop", [NF * 128, D], BF16, kind="ExternalInput")
    wf1p = nc.dram_tensor("wf1p", [64 * 128, D], BF16, kind="ExternalInput")
    wf2p = nc.dram_tensor("wf2p", [NF * 128, FF], BF16, kind="ExternalInput")
    ropeC = nc.dram_tensor("ropeC", [DH, TOK], F32, kind="ExternalInput")
    ropeS2 = nc.dram_tensor("ropeS2", [DH, TOK], F32, kind="ExternalInput")
    maskM = nc.dram_tensor("maskM", [128, 16 * 256], BF16,
                           kind="ExternalInput")
    mbias = nc.dram_tensor("mbias", [128, 16], F32, kind="ExternalInput")
    onesf = nc.dram_tensor("onesf", [128, 1], F32R, kind="ExternalInput")
    onesb = nc.dram_tensor("onesb", [128, 1], BF16, kind="ExternalInput")
    outT = nc.dram_tensor("outT", [D, TOK], F32, kind="ExternalOutput")

    # internal DRAM: AllGather bounce buffers, split in halves so the
    # gathers pipeline on the single CC stream in order k1, v1, k2, v2
    kgin1 = nc.dram_tensor("kgin1", [D // 2, TOK], BF16)   # heads 0-7
    kgout1 = nc.dram_tensor("kgout1", [2 * D, TOK], BF16)
    kgin2 = nc.dram_tensor("kgin2", [D // 2, TOK], BF16)   # heads 8-15
    kgout2 = nc.dram_tensor("kgout2", [2 * D, TOK], BF16)
    vgin1 = nc.dram_tensor("vgin1", [TOK, D // 2], BF16)   # features 0-1023
    vgout1 = nc.dram_tensor("vgout1", [4 * TOK, D // 2], BF16)
    vgin2 = nc.dram_tensor("vgin2", [TOK, D // 2], BF16)   # features 1024-
    vgout2 = nc.dram_tensor("vgout2", [4 * TOK, D // 2], BF16)

    with tile.TileContext(nc) as tc:
        with (
            tc.tile_pool(name="const", bufs=1) as cp,
            tc.tile_pool(name="small", bufs=1) as sp,
            tc.tile_pool(name="mq", bufs=1) as mq,
        ):
            one_r = cp.tile([128, 1], F32R)
            one_b = cp.tile([128, 1], BF16)
            epsc = cp.tile([1, 1], F32)
            nc.scalar.dma_start(one_r[:], onesf[:])
            nc.scalar.dma_start(one_b[:], onesb[:])
            nc.gpsimd.memset(epsc[:], EPS)

            rsB = sp.tile([128, TOK], F32)
            rowS = sp.tile([1, TOK], F32)
            rowR = sp.tile([1, TOK], F32)
            qt = mq.tile([128, NF * TOK], BF16, tag="qt")

            def rmsnorm_rs(ssq_ps):
                # rowR = 1/sqrt(ssq/D + eps), broadcast to 128 partitions
                nc.scalar.activation(rowS[:], ssq_ps[:], AF.Sqrt,
                                     bias=epsc[:], scale=1.0 / D)
                nc.vector.reciprocal(rowR[:], rowS[:])
                nc.gpsimd.partition_broadcast(rsB[:], rowR[:])

            # ====== Phase A-D: norm1, K/V/Q projections, AllGathers ======
            with (
                tc.tile_pool(name="m1", bufs=1) as m1,
                tc.tile_pool(name="ps1", bufs=3, space="PSUM") as ps1,
                tc.tile_pool(name="psr", bufs=1, space="PSUM") as psr,
            ):
                cosT = m1.tile([DH, TOK], F32, tag="cosT")
                sin2 = m1.tile([DH, TOK], F32, tag="sin2")
                nc.scalar.dma_start(cosT[:], ropeC[:])
                nc.scalar.dma_start(sin2[:], ropeS2[:])

                xt = m1.tile([128, NF * TOK], F32, tag="xt")
                xeng = [nc.scalar, nc.gpsimd, nc.scalar, nc.gpsimd]
                for g in range(4):
                    xeng[g].dma_start(
                        xt[:, g * 4 * TOK:(g + 1) * 4 * TOK]
                        .rearrange("p (i c) -> p i c", i=4),
                        xT[g * 512:(g + 1) * 512, :]
                        .rearrange("(i p) c -> p i c", p=128))

                ssq = psr.tile([1, TOK], F32, tag="row")
                for i in range(NF):
                    sq = sp.tile([128, TOK], F32R, tag="sq", bufs=1)
                    nc.scalar.activation(sq[:], xt[:, i * TOK:(i + 1) * TOK],
                                         AF.Square)
                    nc.tensor.matmul(ssq[:], one_r[:], sq[:],
                                     start=(i == 0), stop=(i == NF - 1))
                rmsnorm_rs(ssq)
                xnt = m1.tile([128, NF * TOK], BF16, tag="xn")
                for i in range(NF):
                    nc.vector.tensor_mul(xnt[:, i * TOK:(i + 1) * TOK],
                                         xt[:, i * TOK:(i + 1) * TOK], rsB[:])

                def proj_T(wten, out_tile, rope):
                    """out_tile[:, o*TOK:] = feature-block o of (xn @ w)^T."""
                    for o in range(NF):
                        wc = m1.tile([128, D], BF16, tag="wcol", bufs=4)
                        nc.sync.dma_start(wc[:],
                                          wten[o * 128:(o + 1) * 128, :])
                        acc = ps1.tile([128, TOK], F32, tag="big")
                        for i in range(NF):
                            nc.tensor.matmul(
                                acc[:], wc[:, i * 128:(i + 1) * 128],
                                xnt[:, i * TOK:(i + 1) * TOK],
                                start=(i == 0), stop=(i == NF - 1))
                        dst = out_tile[:, o * TOK:(o + 1) * TOK]
                        if rope:
                            tmp = sp.tile([128, TOK], F32, tag="rtmp",
                                          bufs=1)
                            tmc = sp.tile([128, TOK], F32, tag="rtmc",
                                          bufs=1)
                            nc.vector.tensor_mul(tmp[0:64, :], acc[64:128, :],
                                                 sin2[0:64, :])
                            nc.vector.tensor_mul(tmp[64:128, :], acc[0:64, :],
                                                 sin2[64:128, :])
                            nc.vector.tensor_mul(tmc[:], acc[:], cosT[:])
                            nc.vector.tensor_add(dst, tmc[:], tmp[:])
                        else:
                            nc.vector.tensor_copy(dst, acc[:])
                        yield o

                # K^T (roped) -> kgin chunks -> AllGather #1 (heads 0-7);
                # AllGather #2 is deferred until after the first V gather
                kt = m1.tile([128, NF * TOK], BF16, tag="kt")
                for o in proj_T(wkp, kt, rope=True):
                    kg = kgin1 if o < 8 else kgin2
                    oo = o % 8
                    nc.gpsimd.dma_start(kg.ap()[oo * 128:(oo + 1) * 128, :],
                                        kt[:, o * TOK:(o + 1) * TOK])
                    if o == 7:
                        nc.gpsimd.collective_compute(
                            "AllGather", mybir.AluOpType.bypass,
                            replica_groups=RG4, ins=[kgin1.ap().opt()],
                            outs=[kgout1.ap().opt()])

                # V (token-major, 512-wide feature chunks) -> 2 AllGathers
                vt = m1.tile([128, 4 * D], BF16, tag="vt")
                for fo in range(4):
                    wvc = m1.tile([128, NF * 512], BF16, tag="wv", bufs=2)
                    nc.sync.dma_start(wvc[:],
                                      wvp[fo * 128:(fo + 1) * 128, :])
                    for to in range(4):
                        acc = ps1.tile([128, TOK], F32, tag="vacc", bufs=3)
                        for i in range(NF):
                            nc.tensor.matmul(
                                acc[:],
                                xnt[:, i * TOK + to * 128:
                                    i * TOK + (to + 1) * 128],
                                wvc[:, i * 512:(i + 1) * 512],
                                start=(i == 0), stop=(i == NF - 1))
                        nc.vector.tensor_copy(
                            vt[:, to * D + fo * 512:to * D + (fo + 1) * 512],
                            acc[:])
                    if fo == 1 or fo == 3:
                        vg, base = (vgin1, 0) if fo == 1 else (vgin2, 1024)
                        for to in range(4):
                            nc.gpsimd.dma_start(
                                vg.ap()[to * 128:(to + 1) * 128, :],
                                vt[:, to * D + base:to * D + base + 1024])
                        nc.gpsimd.collective_compute(
                            "AllGather", mybir.AluOpType.bypass,
                            replica_groups=RG4,
                            ins=[vg.ap().opt()],
                            outs=[(vgout1 if fo == 1 else vgout2).ap().opt()])
                    if fo == 1:
                        nc.gpsimd.collective_compute(
                            "AllGather", mybir.AluOpType.bypass,
                            replica_groups=RG4, ins=[kgin2.ap().opt()],
                            outs=[kgout2.ap().opt()])

                # Q^T (roped), stays local in qt
                for _ in proj_T(wqp, qt, rope=True):
                    pass

            # ============ Phase E: attention (local queries) =============
            with tc.tile_pool(name="mo", bufs=1) as mo:
                osb = mo.tile([128, NF * TOK], BF16, tag="osb")
                with (
                    tc.tile_pool(name="m2", bufs=1) as m2,
                    tc.tile_pool(name="ps_s", bufs=3, space="PSUM") as ps_s,
                    tc.tile_pool(name="ps_av", bufs=2, space="PSUM") as ps_av,
                    tc.tile_pool(name="ps_dn", bufs=2, space="PSUM") as ps_dn,
                ):
                    mkT = m2.tile([128, 16 * 256], BF16, tag="mkT")
                    mbT = m2.tile([128, 16], F32, tag="mbT")
                    nc.scalar.dma_start(mkT[:], maskM[:])
                    nc.scalar.dma_start(mbT[:], mbias[:])

                    # gathered K/V streamed per 4-head group: groups 0-1
                    # (heads 0-7) load on scalar before the exps; groups 2-3
                    # on sync after the weight streams, gated by AG k2/v2
                    def load_group(g, eng):
                        ksg = m2.tile([128, 4 * 4 * TOK], BF16, tag="ksbg",
                                      bufs=3)
                        vsg = m2.tile([128, 16 * TOK], BF16, tag="vsbg",
                                      bufs=3)
                        kgo = kgout1 if g < 2 else kgout2
                        vgo = vgout1 if g < 2 else vgout2
                        ho = (g % 2) * 4 * 128        # head offset in half
                        ksgv = ksg[:].rearrange("d (h r c) -> d h r c",
                                                h=4, r=4)
                        vsgv = vsg[:].rearrange("p (G f) -> p G f", G=16)
                        for r in range(4):
                            eng.dma_start(
                                ksgv[:, :, r, :],
                                kgo.ap()[r * 1024 + ho:r * 1024 + ho + 512, :]
                                .rearrange("(h d) c -> d h c", d=128))
                            eng.dma_start(
                                vsgv[:, r * 4:(r + 1) * 4, :],
                                vgo.ap()[r * TOK:(r + 1) * TOK,
                                         ho:ho + 512]
                                .rearrange("(G p) f -> p G f", p=128))
                        return ksg, vsg

                    kvg = {}
                    kvg[0] = load_group(0, nc.scalar)
                    kvg[1] = load_group(1, nc.scalar)
                    kvg[2] = load_group(2, nc.sync)
                    kvg[3] = load_group(3, nc.sync)

                    PIPE = 3
                    for h in range(H):
                        ksg, vsg = kvg[h // 4]
                        hh = h % 4
                        for s in range(2):
                            qs = qt[:, h * TOK + s * 256:
                                    h * TOK + (s + 1) * 256]
                            n_it = 8 if s == 0 else 16
                            av = ps_av.tile([128, 256], F32, tag="av")
                            dn = ps_dn.tile([1, 256], F32, tag="dn")
                            pts = [None] * n_it
                            # software pipeline: scores+exp run PIPE iters
                            # ahead of the dn/av accumulation matmuls
                            for u in range(n_it + PIPE):
                                if u < n_it:
                                    t = u
                                    r, sl, hf = _kv_loc(t)
                                    kcol = (hh * 4 * TOK + r * TOK + sl * 256
                                            + hf * 128)
                                    st = ps_s.tile([128, 256], F32, tag="st")
                                    nc.tensor.matmul(
                                        st[:], ksg[:, kcol:kcol + 128], qs,
                                        start=True, stop=True)
                                    pt = sp.tile([128, 256], BF16, tag="pt",
                                                 bufs=PIPE + 2)
                                    nc.scalar.activation(
                                        pt[:], st[:], AF.Exp,
                                        bias=mbT[:, t:t + 1], scale=ISQ)
                                    if s == 0 or t >= 8:
                                        nc.vector.tensor_mul(
                                            pt[:], pt[:],
                                            mkT[:, t * 256:(t + 1) * 256])
                                    pts[t] = pt
                                if u >= PIPE:
                                    t = u - PIPE
                                    r, sl, hf = _kv_loc(t)
                                    G = r * 4 + sl * 2 + hf
                                    nc.tensor.matmul(dn[:], one_b[:],
                                                     pts[t][:],
                                                     start=(t == 0),
                                                     stop=(t == n_it - 1))
                                    nc.tensor.matmul(
                                        av[:], vsg[:, G * TOK + hh * 128:
                                                   G * TOK + hh * 128 + 128],
                                        pts[t][:], start=(t == 0),
                                        stop=(t == n_it - 1))
                            dnr = sp.tile([1, 256], F32, tag="dnr", bufs=2)
                            nc.scalar.activation(dnr[:], dn[:], AF.Copy)
                            dnB = sp.tile([128, 256], F32, tag="dnB", bufs=2)
                            nc.gpsimd.partition_broadcast(dnB[:], dnr[:])
                            rdB = sp.tile([128, 256], F32, tag="rdB", bufs=2)
                            nc.vector.reciprocal(rdB[:], dnB[:])
                            nc.vector.tensor_mul(
                                osb[:, h * TOK + s * 256:
                                    h * TOK + (s + 1) * 256],
                                av[:], rdB[:])

                # ===== Phase F: O-projection + residual + norm2 stats ====
                with (
                    tc.tile_pool(name="m3", bufs=1) as m3,
                    tc.tile_pool(name="ps3", bufs=3, space="PSUM") as ps3,
                    tc.tile_pool(name="psr3", bufs=1, space="PSUM") as psr3,
                ):
                    x2 = m3.tile([128, NF * TOK], F32, tag="x2")
                    ssq2 = psr3.tile([1, TOK], F32, tag="row")
                    for o in range(NF):
                        wc = m3.tile([128, D], BF16, tag="wocol", bufs=2)
                        nc.sync.dma_start(wc[:],
                                          wop[o * 128:(o + 1) * 128, :])
                        acc = ps3.tile([128, TOK], F32, tag="big")
                        for i in range(NF):
                            nc.tensor.matmul(
                                acc[:], wc[:, i * 128:(i + 1) * 128],
                                osb[:, i * TOK:(i + 1) * TOK],
                                start=(i == 0), stop=(i == NF - 1))
                        xsl = m3.tile([128, TOK], F32, tag="xsl", bufs=2)
                        nc.scalar.dma_start(xsl[:],
                                            xT[o * 128:(o + 1) * 128, :])
                        nc.vector.tensor_add(x2[:, o * TOK:(o + 1) * TOK],
                                             xsl[:], acc[:])
                        sq = sp.tile([128, TOK], F32R, tag="sq", bufs=1)
                        nc.scalar.activation(sq[:],
                                             x2[:, o * TOK:(o + 1) * TOK],
                                             AF.Square)
                        nc.tensor.matmul(ssq2[:], one_r[:], sq[:],
                                         start=(o == 0), stop=(o == NF - 1))
                    rmsnorm_rs(ssq2)

                    # ================ Phase G/H: norm2 + FFN =============
                    with (
                        tc.tile_pool(name="m4", bufs=1) as m4,
                        tc.tile_pool(name="ps4", bufs=3, space="PSUM") as ps4,
                    ):
                        xn2 = m4.tile([128, NF * TOK], BF16, tag="xn2")
                        for i in range(NF):
                            nc.vector.tensor_mul(
                                xn2[:, i * TOK:(i + 1) * TOK],
                                x2[:, i * TOK:(i + 1) * TOK], rsB[:])
                        # ff1 + silu -> hb (bf16, SBUF resident)
                        hb = m4.tile([128, 64 * TOK], BF16, tag="hb")
                        for o in range(FF // 128):
                            wc = m4.tile([128, D], BF16, tag="wf1c", bufs=2)
                            nc.sync.dma_start(
                                wc[:], wf1p[o * 128:(o + 1) * 128, :])
                            acc = ps4.tile([128, TOK], F32, tag="big")
                            for i in range(NF):
                                nc.tensor.matmul(
                                    acc[:], wc[:, i * 128:(i + 1) * 128],
                                    xn2[:, i * TOK:(i + 1) * TOK],
                                    start=(i == 0), stop=(i == NF - 1))
                            nc.scalar.activation(hb[:, o * TOK:(o + 1) * TOK],
                                                 acc[:], AF.Silu)
                        # ff2 + residual -> outT (weights in 1MB half-chunks)
                        for o in range(NF):
                            acc = ps4.tile([128, TOK], F32, tag="big")
                            for half in range(2):
                                wc2 = m4.tile([128, 32 * 128], BF16,
                                              tag="wf2c", bufs=2)
                                nc.sync.dma_start(
                                    wc2[:],
                                    wf2p[o * 128:(o + 1) * 128,
                                         half * 4096:(half + 1) * 4096])
                                for kk in range(32):
                                    k = half * 32 + kk
                                    nc.tensor.matmul(
                                        acc[:],
                                        wc2[:, kk * 128:(kk + 1) * 128],
                                        hb[:, k * TOK:(k + 1) * TOK],
                                        start=(k == 0), stop=(k == 63))
                            osl = m4.tile([128, TOK], F32, tag="osl", bufs=2)
                            nc.vector.tensor_add(osl[:],
                                                 x2[:, o * TOK:(o + 1) * TOK],
                                                 acc[:])
                            nc.sync.dma_start(outT[o * 128:(o + 1) * 128, :],
                                              osl[:])

    nc.compile()
    return nc


_COMPILED = None


def _own_positions(c):
    c4 = c % 4
    a0, a1 = c4, 7 - c4
    pos = np.r_[256 * a0:256 * a0 + 256, 256 * a1:256 * a1 + 256]
    return pos, a0, a1


def _pack_cols(w, oc):
    """[K, M] -> chunk-major [no*128, nk*oc]: rows o*128+p hold, for output
    column chunk o, the weight rows (i*128+p, o-chunk) laid out (i, m)."""
    k, m = w.shape
    nk, no = k // 128, m // oc
    out = np.empty((no * 128, nk * oc), w.dtype)
    for o in range(no):
        c = w[:, o * oc:(o + 1) * oc].reshape(nk, 128, oc)
        out[o * 128:(o + 1) * 128, :] = (
            c.transpose(1, 0, 2).reshape(128, nk * oc))
    return out


def _prep_inmaps(x, rope_cos, rope_sin, mask, w_norm1, w_norm2, wq, wk, wv,
                 wo, w_ff1, w_ff2):
    x = np.asarray(x, np.float32)
    cos = np.asarray(rope_cos, np.float32)
    sin = np.asarray(rope_sin, np.float32)
    mask = np.asarray(mask)
    wn1 = np.asarray(w_norm1, np.float32)[:, None]
    wn2 = np.asarray(w_norm2, np.float32)[:, None]

    bf = ml_dtypes.bfloat16
    wqp = _pack_cols((wn1 * np.asarray(wq, np.float32)).astype(bf), 128)
    wkp = _pack_cols((wn1 * np.asarray(wk, np.float32)).astype(bf), 128)
    wvp = _pack_cols((wn1 * np.asarray(wv, np.float32)).astype(bf), 512)
    wop = _pack_cols(np.asarray(wo, np.float32).astype(bf), 128)
    wf1p = _pack_cols((wn2 * np.asarray(w_ff1, np.float32)).astype(bf), 128)
    wf2p = _pack_cols(np.asarray(w_ff2, np.float32).astype(bf), 128)

    rr = np.arange(128)[:, None]
    cc = np.arange(256)[None, :]
    diag_lo = (cc >= rr).astype(np.float32)
    diag_hi = (cc >= rr + 128).astype(np.float32)
    zeros = np.zeros((128, 256), np.float32)
    ones = np.ones((128, 256), np.float32)

    in_maps = []
    for c in range(NCORES):
        b = c // 4
        pos, a0, a1 = _own_positions(c)
        s = sin[pos].T.copy()
        s2 = np.concatenate([-s[:64], s[64:]], axis=0)

        # multiplicative mask tiles: t<8 -> slot 0 (q-block a0), else slot 1
        tiles = []
        for t in range(16):
            a = a0 if t < 8 else a1
            ext = 2 * a + 2
            if t == ext - 2:
                tiles.append(diag_lo)
            elif t == ext - 1:
                tiles.append(diag_hi)
            elif t >= ext:
                tiles.append(zeros)
            else:
                tiles.append(ones)
        maskM = np.concatenate(tiles, axis=1).astype(bf)

        # key-padding bias per kv block: col j <- kv pos 128*j + r
        mb = np.where(mask[b] != 0, 0.0, NEG).astype(np.float32)  # [L]
        mbias = np.ascontiguousarray(mb.reshape(16, 128).T)

        in_maps.append({
            "xT": np.ascontiguousarray(x[b, pos].T),
            "wqp": wqp, "wkp": wkp, "wvp": wvp, "wop": wop,
            "wf1p": wf1p, "wf2p": wf2p,
            "ropeC": np.ascontiguousarray(cos[pos].T),
            "ropeS2": np.ascontiguousarray(s2),
            "maskM": np.ascontiguousarray(maskM),
            "mbias": mbias,
            "onesf": np.ones((128, 1), np.float32),
            "onesb": np.ones((128, 1), ml_dtypes.bfloat16),
        })
    return in_maps


def _assemble(res):
    out = np.empty((B, L, D), np.float32)
    for c in range(NCORES):
        b = c // 4
        pos, _, _ = _own_positions(c)
        out[b, pos, :] = res.results[c]["outT"].T
    return out


def kernel(**inputs):
    global _COMPILED
    if _COMPILED is None:
        _COMPILED = _build()
    in_maps = _prep_inmaps(**inputs)
    res = run_bass_kernel_spmd(_COMPILED, in_maps, list(range(NCORES)))
    return _assemble(res)


def timed_run(**inputs):
    """Run with NTFF profiling; returns (exec_time_ns, BassKernelResults)."""
    global _COMPILED
    if _COMPILED is None:
        _COMPILED = _build()
    in_maps = _prep_inmaps(**inputs)
    res = run_bass_kernel_spmd(_COMPILED, in_maps, list(range(NCORES)),
                               trace=True)
    return res.exec_time_ns, res
